# revision 10
# baseline (speedup 1.0000x reference)
"""Trainium2 Bass kernel for ModalityAwareDualAttention (dense_cnn).

Sharding: pure data-parallel over batch (32 -> 4 per core x 8 cores).
Per core: loop over P=3 parts; per part 2 pairs of local batches.

Key restructurings vs the reference (exact up to fp assoc.):
  - depthwise scale/bias + avg-pool 0.25 folded into Wq/Wk (+biases)
  - v computed transposed (vT = xd^T @ Wv^T); v-bias commutes through
    softmax (rows sum to 1) and bilinear upsample (rows sum to 1), so
    up = upsample(attn@v) = vT^T @ (attn_n @ KbT) + vb
  - SE global-avg-pool computed from row-sums of xp and of the upsampled
    attention output (accum_out side-channels); SE gate + modality gate
    fused into per-channel affine: out = xp*cw1 + up*cw2
Precision: x/out and residual path bf16; the attention path (v, q/k,
upsample operand) and SE weights fp8 e4m3 with power-of-2 scale folding
(the attention path contributes ~1.4% of output norm, so fp8 error is
strongly attenuated). Matmuls use fp8 DoubleRow (2 k-tiles per pass).
"""

import numpy as np
import ml_dtypes

import concourse.bass as bass
import concourse.tile as tile
import concourse.mybir as mybir

F32 = mybir.dt.float32
BF16 = mybir.dt.bfloat16
F8 = mybir.dt.float8e4
AF = mybir.ActivationFunctionType
ALU = mybir.AluOpType
DR = mybir.MatmulPerfMode.DoubleRow
USE_DR = False              # fp8 DoubleRow matmuls (2 k-tiles per pass)
USE_GPSIMD = True          # offload up*cw2 product to the gpsimd engine

N_CORES = 8
B, C, H, W, P = 32, 2048, 48, 24, 3
BL = B // N_CORES          # 4 local batches per core
IC = 128                   # q/k inter channels
C4 = 512                   # SE bottleneck
PH = H // P                # 16
HD, WD = PH // 2, W // 2   # 8, 12
N = HD * WD                # 96 attention tokens
HWP = PH * W               # 384 spatial positions per part
KC = C // 128              # 16 channel tiles
KT = KC // 2               # 8 DoubleRow k-tile pairs
NP2 = 2 * N                # 192 (pair of batches)

# fp8 scale folding (powers of two)
SQ = 4096.0                # q/k weights+biases scale
SV = 64.0                  # Wv scale
SU = 32.0                  # upsampled-attention (upt) scale
SG = 8.0                   # SE gap scale
SF1 = 64.0                 # fc1 scale
SF2 = 64.0                 # fc2 scale


def _up_matrix(n):
    """[2n, n] bilinear x2 upsample (align_corners=False, edge clamp)."""
    M = np.zeros((2 * n, n), np.float64)
    for o in range(2 * n):
        src = (o + 0.5) / 2.0 - 0.5
        i0 = int(np.floor(src))
        f = src - i0
        M[o, min(max(i0, 0), n - 1)] += 1.0 - f
        M[o, min(max(i0 + 1, 0), n - 1)] += f
    return M


def k_bilinear():
    """[384, 96] upsample matrix: flat(16,24) <- flat(8,12)."""
    return np.kron(_up_matrix(HD), _up_matrix(WD))


def split_excess_waits(nc, max_waits=1):
    """This walrus build rejects multi-sem-wait instructions on some opcodes;
    hoist extra waits onto preceding same-engine no-ops."""
    for f in nc.m.functions:
        for bb in f.blocks:
            insts = bb.instructions
            i = 0
            while i < len(insts):
                ins = insts[i]
                si = ins.sync_info
                if si is not None and si.on_wait and len(si.on_wait) > max_waits:
                    waits = list(si.on_wait)
                    extra, keep = waits[:-max_waits], waits[-max_waits:]
                    nops = []
                    for s in range(0, len(extra), max_waits):
                        nops.append(mybir.InstNoOp(
                            name=nc.get_next_instruction_name(),
                            engine=ins.engine, ins=[], outs=[],
                            sync_info=mybir.SyncInfo(
                                on_wait=extra[s:s + max_waits], on_update=[]),
                        ))
                    ins.sync_info = mybir.SyncInfo(
                        on_wait=keep, on_update=list(si.on_update or []))
                    insts[i:i] = nops
                    i += len(nops)
                i += 1


def build_program(split_waits=True):
    from contextlib import ExitStack
    nc = bass.Bass()

    xh = nc.dram_tensor("xh", [C, P, BL * HWP], BF16, kind="ExternalInput")
    wvd = nc.dram_tensor("wvd", [P, KT, 128, 2, C], F8, kind="ExternalInput")
    wqkd = nc.dram_tensor("wqkd", [P, 2, 128, KT, 2, IC], F8,
                          kind="ExternalInput")
    ktd = nc.dram_tensor("ktd", [P, N, HWP], BF16, kind="ExternalInput")
    fc1d = nc.dram_tensor("fc1d", [P, 128, KC, C4], F8, kind="ExternalInput")
    fc2d = nc.dram_tensor("fc2d", [P, 128, 4, C], F8, kind="ExternalInput")
    # consts cols: 0:16 vbg*SU | 16:32 b2 | 32:36 b1*SG*SF1 | 36 qb*SQ | 37 kb*SQ
    consts = nc.dram_tensor("consts", [P, 128, 38], F32, kind="ExternalInput")
    # gg rows: 0 mwc | 1 mw/SU | 2 mwc/SU  (cols = kc*2 + j)
    gg = nc.dram_tensor("gg", [P, 128, 2, 3, 32], F32, kind="ExternalInput")
    outh = nc.dram_tensor("outh", [C, P, BL * HWP], BF16, kind="ExternalOutput")

    sE = 1.0 / (SQ * SQ)

    with ExitStack() as ctx:
        tc = ctx.enter_context(tile.TileContext(nc))
        pool = lambda name, bufs, **kw: ctx.enter_context(
            tc.tile_pool(name=name, bufs=bufs, **kw))
        wv_pool = pool("wv", 12)
        wqk_pool = pool("wqk", 4)
        fc1_pool = pool("fc1", 2)
        fc2_pool = pool("fc2", 2)
        kt_pool = pool("ktp", 2)
        co_pool = pool("co", 2)
        gg_pool = pool("ggp", 2)
        xp_pool = pool("xp", 32)
        xd_pool = pool("xd", 16)
        t1_pool = pool("t1", 4)
        xs_pool = pool("xs", 36)
        us_pool = pool("us", 36)
        qk_pool = pool("qk", 4)
        attn_pool = pool("attn", 4)
        g_pool = pool("gg2", 2)
        vt_pool = pool("vt", 2)
        up_pool = pool("ups", 34)
        gap_pool = pool("gap", 20)
        se_pool = pool("se", 16)
        uc_pool = pool("uc", 4)
        ot_pool = pool("ot", 4)
        sm_pool = pool("sm", 12)
        ps_vt = pool("ps_vt", 1, space="PSUM")
        ps_bank = pool("ps_bank", 4, space="PSUM")

        for p in range(P):
            # ---------- per-part small loads (issued before x of pair 0) ----
            wq_t = wqk_pool.tile([128, KT, 2, IC], F8, tag="wqk")
            nc.sync.dma_start(wq_t[:], wqkd.ap()[p, 0])
            wk_t = wqk_pool.tile([128, KT, 2, IC], F8, tag="wqk")
            nc.sync.dma_start(wk_t[:], wqkd.ap()[p, 1])
            kt_t = kt_pool.tile([N, HWP], BF16, tag="kt")
            nc.sync.dma_start(kt_t[:], ktd.ap()[p])
            co_t = co_pool.tile([128, 38], F32, tag="co")
            nc.sync.dma_start(co_t[:], consts.ap()[p])
            gg_t = gg_pool.tile([128, 2, 3, 32], F32, tag="ggt")
            nc.sync.dma_start(gg_t[:], gg.ap()[p])

            wv_t = [None] * KT
            fc1_t = fc2_t = None

            for pr in range(2):
                # ---------- load x pair, 2x2-sum-pool -> xd (+ row sums) ----
                xp_t, xs_t = [], []
                xd_t = [xd_pool.tile([128, 2, NP2], F8, tag="xd", name=f"xd{k}")
                        for k in range(KT)]
                for kc in range(KC):
                    xt = xp_pool.tile([128, 2 * HWP], BF16, tag="xp")
                    nc.sync.dma_start(
                        xt[:], xh.ap()[kc * 128:(kc + 1) * 128, p,
                                       pr * 2 * HWP:(pr + 1) * 2 * HWP])
                    xp_t.append(xt)
                    xv4 = xt[:].rearrange("q (j h w) -> q j h w", j=2, h=PH)
                    t1 = t1_pool.tile([128, 2, HD, W], BF16, tag="t1")
                    nc.vector.tensor_tensor(
                        t1[:], xv4[:, :, 0:PH:2, :], xv4[:, :, 1:PH:2, :],
                        ALU.add)
                    t1v = t1[:].rearrange("q j h (w two) -> q j h w two", two=2)
                    xs = xs_pool.tile([128, 2], F32, tag="xs")
                    xs_t.append(xs)
                    for j in range(2):
                        nc.vector.scalar_tensor_tensor(
                            xd_t[kc // 2][:, kc % 2, j * N:(j + 1) * N]
                            .rearrange("q (h w) -> q h w", h=HD),
                            t1v[:, j, :, :, 0], 1.0, t1v[:, j, :, :, 1],
                            ALU.mult, ALU.add, accum_out=xs[:, j:j + 1])

                if pr == 0:
                    # wv/fc DMAs issued after pair-0 x loads so x lands first
                    for kt in range(KT):
                        t = wv_pool.tile([128, 2, C], F8, tag="wv")
                        nc.sync.dma_start(t[:], wvd.ap()[p, kt])
                        wv_t[kt] = t
                    fc1_t = fc1_pool.tile([128, KC, C4], F8, tag="fc1")
                    nc.sync.dma_start(fc1_t[:], fc1d.ap()[p])
                    fc2_t = fc2_pool.tile([128, 4, C], F8, tag="fc2")
                    nc.sync.dma_start(fc2_t[:], fc2d.ap()[p])

                # ---------- q/k projections (DoubleRow, batched pair) ----
                q_ps = ps_bank.tile([IC, NP2], F32, tag="bank")
                if USE_DR:
                    for kt in range(KT):
                        nc.tensor.matmul(q_ps[:], wq_t[:, kt], xd_t[kt][:],
                                         start=(kt == 0), stop=(kt == KT - 1),
                                         perf_mode=DR)
                else:
                    for kc in range(KC):
                        nc.tensor.matmul(q_ps[:], wq_t[:, kc // 2, kc % 2],
                                         xd_t[kc // 2][:, kc % 2],
                                         start=(kc == 0), stop=(kc == KC - 1))
                q_sb = qk_pool.tile([IC, NP2], BF16, tag="qk")
                nc.scalar.activation(q_sb[:], q_ps[:], AF.Identity,
                                     bias=co_t[:, 36:37])
                k_ps = ps_bank.tile([IC, NP2], F32, tag="bank")
                if USE_DR:
                    for kt in range(KT):
                        nc.tensor.matmul(k_ps[:], wk_t[:, kt], xd_t[kt][:],
                                         start=(kt == 0), stop=(kt == KT - 1),
                                         perf_mode=DR)
                else:
                    for kc in range(KC):
                        nc.tensor.matmul(k_ps[:], wk_t[:, kc // 2, kc % 2],
                                         xd_t[kc // 2][:, kc % 2],
                                         start=(kc == 0), stop=(kc == KC - 1))
                k_sb = qk_pool.tile([IC, NP2], BF16, tag="qk")
                nc.scalar.activation(k_sb[:], k_ps[:], AF.Identity,
                                     bias=co_t[:, 37:38])

                us_t = [us_pool.tile([128, 2], F32, tag="us", name=f"us{k}")
                        for k in range(KC)]
                upt_t = {}
                for j in range(2):
                    # ---------- attention (softmax without max-shift:
                    # |energy| ~ 1e-3, exp cannot overflow) ----------
                    e_ps = ps_bank.tile([N, N], F32, tag="bank")
                    nc.tensor.matmul(e_ps[:], q_sb[:, j * N:(j + 1) * N],
                                     k_sb[:, j * N:(j + 1) * N],
                                     start=True, stop=True)
                    attn_e = attn_pool.tile([N, N], BF16, tag="attn")
                    s_sum = sm_pool.tile([N, 1], F32, tag="sm")
                    nc.scalar.activation(attn_e[:], e_ps[:], AF.Exp,
                                         scale=sE, accum_out=s_sum[:])
                    r_sum = sm_pool.tile([N, 1], F32, tag="sm")
                    nc.vector.reciprocal(r_sum[:], s_sum[:])
                    attn_n = attn_pool.tile([N, N], BF16, tag="attn")
                    nc.vector.tensor_scalar(attn_n[:], attn_e[:], r_sum[:],
                                            None, ALU.mult)
                    # ---------- G = attn_n^T @ KbT  [N, 384] ----------
                    g_ps = ps_bank.tile([N, HWP], F32, tag="bank")
                    nc.tensor.matmul(g_ps[:], attn_n[:], kt_t[:],
                                     start=True, stop=True)
                    g_sb = g_pool.tile([N, HWP], BF16, tag="g")
                    nc.scalar.activation(g_sb[:], g_ps[:], AF.Copy)
                    # ---------- vT = xd_b^T @ WvT (DoubleRow) [N, C] ----
                    vt_ps = ps_vt.tile([N, C], F32, tag="vt")
                    if USE_DR:
                        for kt in range(KT):
                            xdb = xd_t[kt][:, :, j * N:(j + 1) * N]
                            for bk in range(4):
                                nc.tensor.matmul(
                                    vt_ps[:, bk * 512:(bk + 1) * 512], xdb,
                                    wv_t[kt][:, :, bk * 512:(bk + 1) * 512],
                                    start=(kt == 0), stop=(kt == KT - 1),
                                    perf_mode=DR)
                    else:
                        for kc in range(KC):
                            xdb = xd_t[kc // 2][:, kc % 2, j * N:(j + 1) * N]
                            for bk in range(4):
                                nc.tensor.matmul(
                                    vt_ps[:, bk * 512:(bk + 1) * 512], xdb,
                                    wv_t[kc // 2][:, kc % 2,
                                                  bk * 512:(bk + 1) * 512],
                                    start=(kc == 0), stop=(kc == KC - 1))
                    vt_sb = vt_pool.tile([N, C], BF16, tag="vt")
                    for bk in range(4):
                        nc.scalar.activation(
                            vt_sb[:, bk * 512:(bk + 1) * 512],
                            vt_ps[:, bk * 512:(bk + 1) * 512], AF.Copy)
                    # ---------- up = vT^T @ G (+ SU*gamma*vb); row sums ----
                    for kc in range(KC):
                        up_ps = ps_bank.tile([128, HWP], F32, tag="bank")
                        nc.tensor.matmul(
                            up_ps[:], vt_sb[:, kc * 128:(kc + 1) * 128],
                            g_sb[:], start=True, stop=True)
                        upt = up_pool.tile([128, HWP], BF16, tag="ups")
                        nc.scalar.activation(
                            upt[:], up_ps[:], AF.Identity, scale=SU / SV,
                            bias=co_t[:, kc:kc + 1],
                            accum_out=us_t[kc][:, j:j + 1])
                        upt_t[(kc, j)] = upt

                # ---------- SE gate (pair, free dim 2) ----------
                gap_t = []
                for kc in range(KC):
                    gp = sm_pool.tile([128, 2], F32, tag="sm")
                    nc.vector.scalar_tensor_tensor(
                        gp[:], us_t[kc][:], 1.0 / SU, xs_t[kc][:],
                        ALU.mult, ALU.add)
                    ga = gap_pool.tile([128, 2], F8, tag="gap")
                    nc.scalar.activation(ga[:], gp[:], AF.Identity,
                                         scale=SG / HWP)
                    gap_t.append(ga)
                h_ps = [ps_bank.tile([128, 2], F32, tag="bank", name=f"h{m}")
                        for m in range(4)]
                for kc in range(KC):
                    for m in range(4):
                        nc.tensor.matmul(
                            h_ps[m][:], fc1_t[:, kc, m * 128:(m + 1) * 128],
                            gap_t[kc][:], start=(kc == 0), stop=(kc == KC - 1))
                h1_t = []
                for m in range(4):
                    hb = se_pool.tile([128, 2], F8, tag="se")
                    nc.scalar.activation(hb[:], h_ps[m][:], AF.Relu,
                                         bias=co_t[:, 32 + m:33 + m])
                    h1_t.append(hb)
                cw_all = se_pool.tile([128, 32], F32, tag="se")
                for kc in range(KC):
                    c_ps = ps_bank.tile([128, 2], F32, tag="bank")
                    for m in range(4):
                        nc.tensor.matmul(
                            c_ps[:], fc2_t[:, m, kc * 128:(kc + 1) * 128],
                            h1_t[m][:], start=(m == 0), stop=(m == 3))
                    nc.scalar.activation(cw_all[:, 2 * kc:2 * kc + 2], c_ps[:],
                                         AF.Sigmoid, scale=1.0 / (SF2 * SG * SF1),
                                         bias=co_t[:, 16 + kc:17 + kc])
                # cw1 = 1 + mwc*cw ; cw2 = (mw + mwc*cw)/SU
                tmp = se_pool.tile([128, 32], F32, tag="se")
                nc.vector.tensor_tensor(tmp[:], cw_all[:], gg_t[:, pr, 0, :],
                                        ALU.mult)
                cw1 = se_pool.tile([128, 32], F32, tag="se")
                nc.vector.tensor_scalar(cw1[:], tmp[:], 1.0, None, ALU.add)
                tmp2 = se_pool.tile([128, 32], F32, tag="se")
                nc.vector.tensor_tensor(tmp2[:], cw_all[:], gg_t[:, pr, 2, :],
                                        ALU.mult)
                cw2 = se_pool.tile([128, 32], F32, tag="se")
                nc.vector.tensor_tensor(cw2[:], tmp2[:], gg_t[:, pr, 1, :],
                                        ALU.add)

                # ---------- final blend + store ----------
                for kc in range(KC):
                    ot = ot_pool.tile([128, 2 * HWP], BF16, tag="ot")
                    for j in range(2):
                        col = 2 * kc + j
                        upc = uc_pool.tile([128, HWP], BF16, tag="uc")
                        if USE_GPSIMD:
                            nc.gpsimd.tensor_scalar(
                                upc[:], upt_t[(kc, j)][:], cw2[:, col:col + 1],
                                None, ALU.mult)
                        else:
                            nc.scalar.activation(
                                upc[:], upt_t[(kc, j)][:], AF.Identity,
                                scale=cw2[:, col:col + 1])
                        nc.vector.scalar_tensor_tensor(
                            ot[:, j * HWP:(j + 1) * HWP],
                            xp_t[kc][:, j * HWP:(j + 1) * HWP],
                            cw1[:, col:col + 1], upc[:], ALU.mult, ALU.add)
                    nc.sync.dma_start(
                        outh.ap()[kc * 128:(kc + 1) * 128, p,
                                  pr * 2 * HWP:(pr + 1) * 2 * HWP], ot[:])

    if split_waits:
        split_excess_waits(nc)
    return nc


# ---------------------------------------------------------------------------
# Host side
# ---------------------------------------------------------------------------

def _sigmoid(v):
    return 1.0 / (1.0 + np.exp(-v))


def _bf(a):
    return np.ascontiguousarray(np.asarray(a).astype(ml_dtypes.bfloat16))


def _f8(a):
    return np.ascontiguousarray(
        np.clip(np.asarray(a, np.float64), -224.0, 224.0)
        .astype(ml_dtypes.float8_e4m3))


def _f32(a):
    return np.ascontiguousarray(np.asarray(a, dtype=np.float32))


def prepare_host_inputs(inputs):
    """Fold/transpose/scale weights; returns per-core input dicts."""
    g = {k: np.asarray(v) for k, v in inputs.items()}
    x = np.asarray(g["x"], np.float32)

    # modality gate on host (tiny): mw [B, P]
    mf = g["modality"].astype(np.float64)[:, None]
    g1 = np.maximum(mf @ g["gate_w1"].astype(np.float64).T
                    + g["gate_b1"].astype(np.float64), 0.0)
    mw = _sigmoid(g1 @ g["gate_w2"].astype(np.float64).T
                  + g["gate_b2"].astype(np.float64))      # [B, P]

    paq = g["pa_q_w"].astype(np.float64)    # [P, IC, C]
    pak = g["pa_k_w"].astype(np.float64)
    pav = g["pa_v_w"].astype(np.float64)    # [P, C, C]
    dwq_w = g["pa_dw_q_w"].astype(np.float64)   # [P, C]
    dwq_b = g["pa_dw_q_b"].astype(np.float64)
    dwk_w = g["pa_dw_k_w"].astype(np.float64)
    dwk_b = g["pa_dw_k_b"].astype(np.float64)
    gam = g["pa_gamma"].astype(np.float64)      # [P]
    cgam = g["ca_gamma"].astype(np.float64)

    # q/k weights [P, C, IC], scaled by SQ (and 0.25 pool fold)
    wq_full = np.stack([(paq[p] * dwq_w[p][None, :] * 0.25 * SQ).T
                        for p in range(P)])
    wk_full = np.stack([(pak[p] * dwk_w[p][None, :] * 0.25 * SQ).T
                        for p in range(P)])
    qb = np.stack([SQ * (g["pa_q_b"][p] + paq[p] @ dwq_b[p]) for p in range(P)])
    kb = np.stack([SQ * (g["pa_k_b"][p] + pak[p] @ dwk_b[p]) for p in range(P)])
    # DoubleRow interleave [P, 2, 128, KT, 2, IC]
    wqk = np.stack([wq_full, wk_full], axis=1)          # [P, 2, C, IC]
    wqkd = _f8(wqk.reshape(P, 2, KT, 2, 128, IC).transpose(0, 1, 4, 2, 3, 5))

    # v weights [P, C, C] scaled by SV -> [P, KT, 128, 2, C]
    wv_full = np.stack([0.25 * SV * pav[p].T for p in range(P)])
    wvd = _f8(wv_full.reshape(P, KT, 2, 128, C).transpose(0, 1, 3, 2, 4))

    # upsample matrix with pa_gamma folded: [P, N, HWP]
    kb_mat = k_bilinear()                     # [384, 96]
    ktd = _bf(np.stack([gam[p] * kb_mat.T for p in range(P)]))

    fc1 = g["ca_fc1_w"].astype(np.float64)    # [P, C4, C]
    fc2 = g["ca_fc2_w"].astype(np.float64)    # [P, C, C4]
    fc1T = np.stack([SF1 * fc1[p].T for p in range(P)])   # [P, C, C4]
    fc2T = np.stack([SF2 * fc2[p].T for p in range(P)])   # [P, C4, C]
    fc1d = _f8(fc1T.reshape(P, KC, 128, C4).transpose(0, 2, 1, 3))
    fc2d = _f8(fc2T.reshape(P, 4, 128, C).transpose(0, 2, 1, 3))

    consts = np.zeros((P, 128, 38), np.float32)
    for p in range(P):
        vbg = SU * gam[p] * g["pa_v_b"][p].astype(np.float64)     # [C]
        consts[p, :, 0:16] = vbg.reshape(16, 128).T
        consts[p, :, 16:32] = g["ca_fc2_b"][p].astype(np.float64).reshape(
            16, 128).T
        consts[p, :, 32:36] = (SG * SF1 * g["ca_fc1_b"][p].astype(
            np.float64)).reshape(4, 128).T
        consts[p, :, 36] = qb[p]
        consts[p, :, 37] = kb[p]

    shared = {"wvd": wvd, "wqkd": wqkd, "ktd": ktd, "fc1d": fc1d,
              "fc2d": fc2d, "consts": consts}

    per_core = []
    for c in range(N_CORES):
        xs = x[c * BL:(c + 1) * BL]            # [BL, C, H, W]
        xhc = _bf(xs.reshape(BL, C, P, PH, W).transpose(1, 2, 0, 3, 4)
                  .reshape(C, P, BL * PH * W))
        mwl = mw[c * BL:(c + 1) * BL]          # [BL, P]
        ggc = np.zeros((P, 128, 2, 3, 32), np.float32)
        for p in range(P):
            for b in range(BL):
                pr, j = b // 2, b % 2
                cols = np.arange(KC) * 2 + j
                mwc = mwl[b, p] * cgam[p]
                ggc[p, :, pr, 0, cols] = mwc
                ggc[p, :, pr, 1, cols] = mwl[b, p] / SU
                ggc[p, :, pr, 2, cols] = mwc / SU
        per_core.append({"xh": xhc, "gg": ggc, **shared})
    return per_core


_CACHE = {}
TRACE = False
TRACE_DIR = None


def kernel(**inputs):
    from concourse.bass_utils import run_bass_kernel_spmd

    per_core = prepare_host_inputs(inputs)
    if "nc" not in _CACHE:
        _CACHE["nc"] = build_program()
    nc = _CACHE["nc"]
    kw = dict(trace=True, tmpdir=TRACE_DIR) if TRACE else {}
    res = run_bass_kernel_spmd(nc, per_core, list(range(N_CORES)), **kw)
    _CACHE["last_results"] = res
    outs = []
    for c in range(N_CORES):
        oh = np.asarray(res.results[c]["outh"]).astype(np.float32)
        outs.append(oh.reshape(C, P, BL, PH, W).transpose(2, 0, 1, 3, 4)
                    .reshape(BL, C, H, W))
    return np.concatenate(outs, axis=0)


# revision 12
# speedup vs baseline: 2.9180x; 2.9180x over previous
"""Trainium2 Bass kernel for ModalityAwareDualAttention (dense_cnn).

Sharding: pure data-parallel over batch (32 -> 4 per core x 8 cores).
Per core: loop over P=3 parts; per part 2 pairs of local batches.

Key restructurings vs the reference (exact up to fp assoc.):
  - depthwise scale/bias + avg-pool 0.25 folded into Wq/Wk (+biases)
  - v computed transposed (vT = xd^T @ Wv^T); v-bias commutes through
    softmax (rows sum to 1) and bilinear upsample (rows sum to 1), so
    up = upsample(attn@v) = vT^T @ (attn_n @ KbT) + vb
  - SE global-avg-pool computed from row-sums of xp and of the upsampled
    attention output (accum_out side-channels); SE gate + modality gate
    fused into per-channel affine: out = xp*cw1 + up*cw2
Precision: x/out and residual path bf16; the attention path (v, q/k,
upsample operand) and SE weights fp8 e4m3 with power-of-2 scale folding
(the attention path contributes ~1.4% of output norm, so fp8 error is
strongly attenuated). Matmuls use fp8 DoubleRow (2 k-tiles per pass).
"""

import numpy as np
import ml_dtypes

import concourse.bass as bass
import concourse.tile as tile
import concourse.mybir as mybir

F32 = mybir.dt.float32
BF16 = mybir.dt.bfloat16
F8 = mybir.dt.float8e4
AF = mybir.ActivationFunctionType
ALU = mybir.AluOpType
DR = mybir.MatmulPerfMode.DoubleRow
USE_DR = False             # fp8 DoubleRow for the vt matmuls
USE_DR_QK = True           # fp8 DoubleRow for q/k (full 128-col stationary)
USE_GPSIMD = False         # offload up*cw2 product to the gpsimd engine

N_CORES = 8
B, C, H, W, P = 32, 2048, 48, 24, 3
BL = B // N_CORES          # 4 local batches per core
IC = 128                   # q/k inter channels
C4 = 512                   # SE bottleneck
PH = H // P                # 16
HD, WD = PH // 2, W // 2   # 8, 12
N = HD * WD                # 96 attention tokens
HWP = PH * W               # 384 spatial positions per part
KC = C // 128              # 16 channel tiles
KT = KC // 2               # 8 DoubleRow k-tile pairs
NP2 = 2 * N                # 192 (pair of batches)

# fp8 scale folding (powers of two)
SQ = 4096.0                # q/k weights+biases scale
SV = 64.0                  # Wv scale
SU = 32.0                  # upsampled-attention (upt) scale
SG = 8.0                   # SE gap scale
SF1 = 64.0                 # fc1 scale
SF2 = 64.0                 # fc2 scale


def _up_matrix(n):
    """[2n, n] bilinear x2 upsample (align_corners=False, edge clamp)."""
    M = np.zeros((2 * n, n), np.float64)
    for o in range(2 * n):
        src = (o + 0.5) / 2.0 - 0.5
        i0 = int(np.floor(src))
        f = src - i0
        M[o, min(max(i0, 0), n - 1)] += 1.0 - f
        M[o, min(max(i0 + 1, 0), n - 1)] += f
    return M


def k_bilinear():
    """[384, 96] upsample matrix: flat(16,24) <- flat(8,12)."""
    return np.kron(_up_matrix(HD), _up_matrix(WD))


def split_excess_waits(nc, max_waits=1):
    """This walrus build rejects multi-sem-wait instructions on some opcodes;
    hoist extra waits onto preceding same-engine no-ops."""
    for f in nc.m.functions:
        for bb in f.blocks:
            insts = bb.instructions
            i = 0
            while i < len(insts):
                ins = insts[i]
                si = ins.sync_info
                if si is not None and si.on_wait and len(si.on_wait) > max_waits:
                    waits = list(si.on_wait)
                    extra, keep = waits[:-max_waits], waits[-max_waits:]
                    nops = []
                    for s in range(0, len(extra), max_waits):
                        nops.append(mybir.InstNoOp(
                            name=nc.get_next_instruction_name(),
                            engine=ins.engine, ins=[], outs=[],
                            sync_info=mybir.SyncInfo(
                                on_wait=extra[s:s + max_waits], on_update=[]),
                        ))
                    ins.sync_info = mybir.SyncInfo(
                        on_wait=keep, on_update=list(si.on_update or []))
                    insts[i:i] = nops
                    i += len(nops)
                i += 1


def build_program(split_waits=True):
    from contextlib import ExitStack
    nc = bass.Bass()

    xh = nc.dram_tensor("xh", [C, P, BL * HWP], BF16, kind="ExternalInput")
    wvd = nc.dram_tensor("wvd", [P, KT, 128, 2, C], F8, kind="ExternalInput")
    wqkd = nc.dram_tensor("wqkd", [P, 2, 128, KT, 2, IC], F8,
                          kind="ExternalInput")
    ktd = nc.dram_tensor("ktd", [P, N, HWP], BF16, kind="ExternalInput")
    fc1d = nc.dram_tensor("fc1d", [P, 128, KC, C4], F8, kind="ExternalInput")
    fc2d = nc.dram_tensor("fc2d", [P, 128, 4, C], F8, kind="ExternalInput")
    # consts cols: 0:16 vbg*SU | 16:32 b2 | 32:36 b1*SG*SF1 | 36 qb*SQ | 37 kb*SQ
    consts = nc.dram_tensor("consts", [P, 128, 38], F32, kind="ExternalInput")
    # gg rows: 0 mwc | 1 mw/SU | 2 mwc/SU  (cols = kc*2 + j)
    gg = nc.dram_tensor("gg", [P, 128, 2, 3, 32], F32, kind="ExternalInput")
    outh = nc.dram_tensor("outh", [C, P, BL * HWP], BF16, kind="ExternalOutput")

    sE = 1.0 / (SQ * SQ)

    with ExitStack() as ctx:
        tc = ctx.enter_context(tile.TileContext(nc))
        pool = lambda name, bufs, **kw: ctx.enter_context(
            tc.tile_pool(name=name, bufs=bufs, **kw))
        wv_pool = pool("wv", 12)
        wqk_pool = pool("wqk", 4)
        fc1_pool = pool("fc1", 2)
        fc2_pool = pool("fc2", 2)
        kt_pool = pool("ktp", 2)
        co_pool = pool("co", 2)
        gg_pool = pool("ggp", 2)
        xp_pool = pool("xp", 32)
        xd_pool = pool("xd", 16)
        t1_pool = pool("t1", 4)
        xs_pool = pool("xs", 36)
        us_pool = pool("us", 36)
        qk_pool = pool("qk", 4)
        attn_pool = pool("attn", 4)
        g_pool = pool("gg2", 2)
        vt_pool = pool("vt", 2)
        up_pool = pool("ups", 34)
        gap_pool = pool("gap", 20)
        se_pool = pool("se", 16)
        uc_pool = pool("uc", 4)
        ot_pool = pool("ot", 4)
        sm_pool = pool("sm", 12)
        ps_vt = pool("ps_vt", 1, space="PSUM")
        ps_bank = pool("ps_bank", 4, space="PSUM")

        for p in range(P):
            # ---------- per-part small loads (issued before x of pair 0) ----
            wq_t = wqk_pool.tile([128, KT, 2, IC], F8, tag="wqk")
            nc.sync.dma_start(wq_t[:], wqkd.ap()[p, 0])
            wk_t = wqk_pool.tile([128, KT, 2, IC], F8, tag="wqk")
            nc.sync.dma_start(wk_t[:], wqkd.ap()[p, 1])
            kt_t = kt_pool.tile([N, HWP], BF16, tag="kt")
            nc.sync.dma_start(kt_t[:], ktd.ap()[p])
            co_t = co_pool.tile([128, 38], F32, tag="co")
            nc.sync.dma_start(co_t[:], consts.ap()[p])
            gg_t = gg_pool.tile([128, 2, 3, 32], F32, tag="ggt")
            nc.sync.dma_start(gg_t[:], gg.ap()[p])

            wv_t = [None] * KT
            fc1_t = fc2_t = None

            for pr in range(2):
                # ---------- load x pair, 2x2-sum-pool -> xd (+ row sums) ----
                xp_t, xs_t = [], []
                xd_t = [xd_pool.tile([128, 2, NP2], F8, tag="xd", name=f"xd{k}")
                        for k in range(KT)]
                for kc in range(KC):
                    xt = xp_pool.tile([128, 2 * HWP], BF16, tag="xp")
                    nc.sync.dma_start(
                        xt[:], xh.ap()[kc * 128:(kc + 1) * 128, p,
                                       pr * 2 * HWP:(pr + 1) * 2 * HWP])
                    xp_t.append(xt)
                    xv4 = xt[:].rearrange("q (j h w) -> q j h w", j=2, h=PH)
                    t1 = t1_pool.tile([128, 2, HD, W], BF16, tag="t1")
                    nc.vector.tensor_tensor(
                        t1[:], xv4[:, :, 0:PH:2, :], xv4[:, :, 1:PH:2, :],
                        ALU.add)
                    t1v = t1[:].rearrange("q j h (w two) -> q j h w two", two=2)
                    xs = xs_pool.tile([128, 2], F32, tag="xs")
                    xs_t.append(xs)
                    for j in range(2):
                        nc.vector.scalar_tensor_tensor(
                            xd_t[kc // 2][:, kc % 2, j * N:(j + 1) * N]
                            .rearrange("q (h w) -> q h w", h=HD),
                            t1v[:, j, :, :, 0], 1.0, t1v[:, j, :, :, 1],
                            ALU.mult, ALU.add, accum_out=xs[:, j:j + 1])

                if pr == 0:
                    # wv/fc DMAs issued after pair-0 x loads so x lands first
                    for kt in range(KT):
                        t = wv_pool.tile([128, 2, C], F8, tag="wv")
                        nc.sync.dma_start(t[:], wvd.ap()[p, kt])
                        wv_t[kt] = t
                    fc1_t = fc1_pool.tile([128, KC, C4], F8, tag="fc1")
                    nc.sync.dma_start(fc1_t[:], fc1d.ap()[p])
                    fc2_t = fc2_pool.tile([128, 4, C], F8, tag="fc2")
                    nc.sync.dma_start(fc2_t[:], fc2d.ap()[p])

                # ---------- q/k projections (DoubleRow, batched pair) ----
                q_ps = ps_bank.tile([IC, NP2], F32, tag="bank")
                if USE_DR_QK:
                    for kt in range(KT):
                        nc.tensor.matmul(q_ps[:], wq_t[:, kt], xd_t[kt][:],
                                         start=(kt == 0), stop=(kt == KT - 1),
                                         perf_mode=DR)
                else:
                    for kc in range(KC):
                        nc.tensor.matmul(q_ps[:], wq_t[:, kc // 2, kc % 2],
                                         xd_t[kc // 2][:, kc % 2],
                                         start=(kc == 0), stop=(kc == KC - 1))
                q_sb = qk_pool.tile([IC, NP2], BF16, tag="qk")
                nc.scalar.activation(q_sb[:], q_ps[:], AF.Identity,
                                     bias=co_t[:, 36:37])
                k_ps = ps_bank.tile([IC, NP2], F32, tag="bank")
                if USE_DR_QK:
                    for kt in range(KT):
                        nc.tensor.matmul(k_ps[:], wk_t[:, kt], xd_t[kt][:],
                                         start=(kt == 0), stop=(kt == KT - 1),
                                         perf_mode=DR)
                else:
                    for kc in range(KC):
                        nc.tensor.matmul(k_ps[:], wk_t[:, kc // 2, kc % 2],
                                         xd_t[kc // 2][:, kc % 2],
                                         start=(kc == 0), stop=(kc == KC - 1))
                k_sb = qk_pool.tile([IC, NP2], BF16, tag="qk")
                nc.scalar.activation(k_sb[:], k_ps[:], AF.Identity,
                                     bias=co_t[:, 37:38])

                us_t = [us_pool.tile([128, 2], F32, tag="us", name=f"us{k}")
                        for k in range(KC)]
                upt_t = {}
                for j in range(2):
                    # ---------- attention (softmax without max-shift:
                    # |energy| ~ 1e-3, exp cannot overflow) ----------
                    e_ps = ps_bank.tile([N, N], F32, tag="bank")
                    nc.tensor.matmul(e_ps[:], q_sb[:, j * N:(j + 1) * N],
                                     k_sb[:, j * N:(j + 1) * N],
                                     start=True, stop=True)
                    attn_e = attn_pool.tile([N, N], BF16, tag="attn")
                    s_sum = sm_pool.tile([N, 1], F32, tag="sm")
                    nc.scalar.activation(attn_e[:], e_ps[:], AF.Exp,
                                         scale=sE, accum_out=s_sum[:])
                    r_sum = sm_pool.tile([N, 1], F32, tag="sm")
                    nc.vector.reciprocal(r_sum[:], s_sum[:])
                    attn_n = attn_pool.tile([N, N], BF16, tag="attn")
                    nc.vector.tensor_scalar(attn_n[:], attn_e[:], r_sum[:],
                                            None, ALU.mult)
                    # ---------- G = attn_n^T @ KbT  [N, 384] ----------
                    g_ps = ps_bank.tile([N, HWP], F32, tag="bank")
                    nc.tensor.matmul(g_ps[:], attn_n[:], kt_t[:],
                                     start=True, stop=True)
                    g_sb = g_pool.tile([N, HWP], BF16, tag="g")
                    nc.scalar.activation(g_sb[:], g_ps[:], AF.Copy)
                    # ---------- vT = xd_b^T @ WvT (DoubleRow) [N, C] ----
                    vt_ps = ps_vt.tile([N, C], F32, tag="vt")
                    if USE_DR:
                        for kt in range(KT):
                            xdb = xd_t[kt][:, :, j * N:(j + 1) * N]
                            for bk in range(4):
                                nc.tensor.matmul(
                                    vt_ps[:, bk * 512:(bk + 1) * 512], xdb,
                                    wv_t[kt][:, :, bk * 512:(bk + 1) * 512],
                                    start=(kt == 0), stop=(kt == KT - 1),
                                    perf_mode=DR)
                    else:
                        for kc in range(KC):
                            xdb = xd_t[kc // 2][:, kc % 2, j * N:(j + 1) * N]
                            for bk in range(4):
                                nc.tensor.matmul(
                                    vt_ps[:, bk * 512:(bk + 1) * 512], xdb,
                                    wv_t[kc // 2][:, kc % 2,
                                                  bk * 512:(bk + 1) * 512],
                                    start=(kc == 0), stop=(kc == KC - 1))
                    vt_sb = vt_pool.tile([N, C], BF16, tag="vt")
                    for bk in range(4):
                        nc.scalar.activation(
                            vt_sb[:, bk * 512:(bk + 1) * 512],
                            vt_ps[:, bk * 512:(bk + 1) * 512], AF.Copy)
                    # ---------- up = vT^T @ G (+ SU*gamma*vb); row sums ----
                    for kc in range(KC):
                        up_ps = ps_bank.tile([128, HWP], F32, tag="bank")
                        nc.tensor.matmul(
                            up_ps[:], vt_sb[:, kc * 128:(kc + 1) * 128],
                            g_sb[:], start=True, stop=True)
                        upt = up_pool.tile([128, HWP], BF16, tag="ups")
                        nc.scalar.activation(
                            upt[:], up_ps[:], AF.Identity, scale=SU / SV,
                            bias=co_t[:, kc:kc + 1],
                            accum_out=us_t[kc][:, j:j + 1])
                        upt_t[(kc, j)] = upt

                # ---------- SE gate (pair, free dim 2) ----------
                gap_t = []
                for kc in range(KC):
                    gp = sm_pool.tile([128, 2], F32, tag="sm")
                    nc.vector.scalar_tensor_tensor(
                        gp[:], us_t[kc][:], 1.0 / SU, xs_t[kc][:],
                        ALU.mult, ALU.add)
                    ga = gap_pool.tile([128, 2], F8, tag="gap")
                    nc.scalar.activation(ga[:], gp[:], AF.Identity,
                                         scale=SG / HWP)
                    gap_t.append(ga)
                h_ps = [ps_bank.tile([128, 2], F32, tag="bank", name=f"h{m}")
                        for m in range(4)]
                for kc in range(KC):
                    for m in range(4):
                        nc.tensor.matmul(
                            h_ps[m][:], fc1_t[:, kc, m * 128:(m + 1) * 128],
                            gap_t[kc][:], start=(kc == 0), stop=(kc == KC - 1))
                h1_t = []
                for m in range(4):
                    hb = se_pool.tile([128, 2], F8, tag="se")
                    nc.scalar.activation(hb[:], h_ps[m][:], AF.Relu,
                                         bias=co_t[:, 32 + m:33 + m])
                    h1_t.append(hb)
                cw_all = se_pool.tile([128, 32], F32, tag="se")
                for kc in range(KC):
                    c_ps = ps_bank.tile([128, 2], F32, tag="bank")
                    for m in range(4):
                        nc.tensor.matmul(
                            c_ps[:], fc2_t[:, m, kc * 128:(kc + 1) * 128],
                            h1_t[m][:], start=(m == 0), stop=(m == 3))
                    nc.scalar.activation(cw_all[:, 2 * kc:2 * kc + 2], c_ps[:],
                                         AF.Sigmoid, scale=1.0 / (SF2 * SG * SF1),
                                         bias=co_t[:, 16 + kc:17 + kc])
                # cw1 = 1 + mwc*cw ; cw2 = (mw + mwc*cw)/SU
                tmp = se_pool.tile([128, 32], F32, tag="se")
                nc.vector.tensor_tensor(tmp[:], cw_all[:], gg_t[:, pr, 0, :],
                                        ALU.mult)
                cw1 = se_pool.tile([128, 32], F32, tag="se")
                nc.vector.tensor_scalar(cw1[:], tmp[:], 1.0, None, ALU.add)
                tmp2 = se_pool.tile([128, 32], F32, tag="se")
                nc.vector.tensor_tensor(tmp2[:], cw_all[:], gg_t[:, pr, 2, :],
                                        ALU.mult)
                cw2 = se_pool.tile([128, 32], F32, tag="se")
                nc.vector.tensor_tensor(cw2[:], tmp2[:], gg_t[:, pr, 1, :],
                                        ALU.add)

                # ---------- final blend + store ----------
                for kc in range(KC):
                    ot = ot_pool.tile([128, 2 * HWP], BF16, tag="ot")
                    for j in range(2):
                        col = 2 * kc + j
                        r1 = uc_pool.tile([128, HWP], F32, tag="uc")
                        nc.scalar.activation(
                            r1[:], xp_t[kc][:, j * HWP:(j + 1) * HWP],
                            AF.Identity, scale=cw1[:, col:col + 1])
                        nc.vector.scalar_tensor_tensor(
                            ot[:, j * HWP:(j + 1) * HWP], upt_t[(kc, j)][:],
                            cw2[:, col:col + 1], r1[:], ALU.mult, ALU.add)
                    nc.sync.dma_start(
                        outh.ap()[kc * 128:(kc + 1) * 128, p,
                                  pr * 2 * HWP:(pr + 1) * 2 * HWP], ot[:])

    if split_waits:
        split_excess_waits(nc)
    return nc


# ---------------------------------------------------------------------------
# Host side
# ---------------------------------------------------------------------------

def _sigmoid(v):
    return 1.0 / (1.0 + np.exp(-v))


def _bf(a):
    return np.ascontiguousarray(np.asarray(a).astype(ml_dtypes.bfloat16))


def _f8(a):
    return np.ascontiguousarray(
        np.clip(np.asarray(a, np.float64), -224.0, 224.0)
        .astype(ml_dtypes.float8_e4m3))


def _f32(a):
    return np.ascontiguousarray(np.asarray(a, dtype=np.float32))


def prepare_host_inputs(inputs):
    """Fold/transpose/scale weights; returns per-core input dicts."""
    g = {k: np.asarray(v) for k, v in inputs.items()}
    x = np.asarray(g["x"], np.float32)

    # modality gate on host (tiny): mw [B, P]
    mf = g["modality"].astype(np.float64)[:, None]
    g1 = np.maximum(mf @ g["gate_w1"].astype(np.float64).T
                    + g["gate_b1"].astype(np.float64), 0.0)
    mw = _sigmoid(g1 @ g["gate_w2"].astype(np.float64).T
                  + g["gate_b2"].astype(np.float64))      # [B, P]

    paq = g["pa_q_w"].astype(np.float64)    # [P, IC, C]
    pak = g["pa_k_w"].astype(np.float64)
    pav = g["pa_v_w"].astype(np.float64)    # [P, C, C]
    dwq_w = g["pa_dw_q_w"].astype(np.float64)   # [P, C]
    dwq_b = g["pa_dw_q_b"].astype(np.float64)
    dwk_w = g["pa_dw_k_w"].astype(np.float64)
    dwk_b = g["pa_dw_k_b"].astype(np.float64)
    gam = g["pa_gamma"].astype(np.float64)      # [P]
    cgam = g["ca_gamma"].astype(np.float64)

    # q/k weights [P, C, IC], scaled by SQ (and 0.25 pool fold)
    wq_full = np.stack([(paq[p] * dwq_w[p][None, :] * 0.25 * SQ).T
                        for p in range(P)])
    wk_full = np.stack([(pak[p] * dwk_w[p][None, :] * 0.25 * SQ).T
                        for p in range(P)])
    qb = np.stack([SQ * (g["pa_q_b"][p] + paq[p] @ dwq_b[p]) for p in range(P)])
    kb = np.stack([SQ * (g["pa_k_b"][p] + pak[p] @ dwk_b[p]) for p in range(P)])
    # DoubleRow interleave [P, 2, 128, KT, 2, IC]
    wqk = np.stack([wq_full, wk_full], axis=1)          # [P, 2, C, IC]
    wqkd = _f8(wqk.reshape(P, 2, KT, 2, 128, IC).transpose(0, 1, 4, 2, 3, 5))

    # v weights [P, C, C] scaled by SV -> [P, KT, 128, 2, C]
    wv_full = np.stack([0.25 * SV * pav[p].T for p in range(P)])
    wvd = _f8(wv_full.reshape(P, KT, 2, 128, C).transpose(0, 1, 3, 2, 4))

    # upsample matrix with pa_gamma folded: [P, N, HWP]
    kb_mat = k_bilinear()                     # [384, 96]
    ktd = _bf(np.stack([gam[p] * kb_mat.T for p in range(P)]))

    fc1 = g["ca_fc1_w"].astype(np.float64)    # [P, C4, C]
    fc2 = g["ca_fc2_w"].astype(np.float64)    # [P, C, C4]
    fc1T = np.stack([SF1 * fc1[p].T for p in range(P)])   # [P, C, C4]
    fc2T = np.stack([SF2 * fc2[p].T for p in range(P)])   # [P, C4, C]
    fc1d = _f8(fc1T.reshape(P, KC, 128, C4).transpose(0, 2, 1, 3))
    fc2d = _f8(fc2T.reshape(P, 4, 128, C).transpose(0, 2, 1, 3))

    consts = np.zeros((P, 128, 38), np.float32)
    for p in range(P):
        vbg = SU * gam[p] * g["pa_v_b"][p].astype(np.float64)     # [C]
        consts[p, :, 0:16] = vbg.reshape(16, 128).T
        consts[p, :, 16:32] = g["ca_fc2_b"][p].astype(np.float64).reshape(
            16, 128).T
        consts[p, :, 32:36] = (SG * SF1 * g["ca_fc1_b"][p].astype(
            np.float64)).reshape(4, 128).T
        consts[p, :, 36] = qb[p]
        consts[p, :, 37] = kb[p]

    shared = {"wvd": wvd, "wqkd": wqkd, "ktd": ktd, "fc1d": fc1d,
              "fc2d": fc2d, "consts": consts}

    per_core = []
    for c in range(N_CORES):
        xs = x[c * BL:(c + 1) * BL]            # [BL, C, H, W]
        xhc = _bf(xs.reshape(BL, C, P, PH, W).transpose(1, 2, 0, 3, 4)
                  .reshape(C, P, BL * PH * W))
        mwl = mw[c * BL:(c + 1) * BL]          # [BL, P]
        ggc = np.zeros((P, 128, 2, 3, 32), np.float32)
        for p in range(P):
            for b in range(BL):
                pr, j = b // 2, b % 2
                cols = np.arange(KC) * 2 + j
                mwc = mwl[b, p] * cgam[p]
                ggc[p, :, pr, 0, cols] = mwc
                ggc[p, :, pr, 1, cols] = mwl[b, p] / SU
                ggc[p, :, pr, 2, cols] = mwc / SU
        per_core.append({"xh": xhc, "gg": ggc, **shared})
    return per_core


_CACHE = {}
TRACE = False
TRACE_DIR = None


def kernel(**inputs):
    from concourse.bass_utils import run_bass_kernel_spmd

    per_core = prepare_host_inputs(inputs)
    if "nc" not in _CACHE:
        _CACHE["nc"] = build_program()
    nc = _CACHE["nc"]
    kw = dict(trace=True, tmpdir=TRACE_DIR) if TRACE else {}
    res = run_bass_kernel_spmd(nc, per_core, list(range(N_CORES)), **kw)
    _CACHE["last_results"] = res
    outs = []
    for c in range(N_CORES):
        oh = np.asarray(res.results[c]["outh"]).astype(np.float32)
        outs.append(oh.reshape(C, P, BL, PH, W).transpose(2, 0, 1, 3, 4)
                    .reshape(BL, C, H, W))
    return np.concatenate(outs, axis=0)


# revision 13
# speedup vs baseline: 3.2103x; 1.1002x over previous
"""Trainium2 Bass kernel for ModalityAwareDualAttention (dense_cnn).

Sharding: pure data-parallel over batch (32 -> 4 per core x 8 cores).
Per core: loop over P=3 parts; per part 2 pairs of local batches.

Key restructurings vs the reference (exact up to fp assoc.):
  - depthwise scale/bias + avg-pool 0.25 folded into Wq/Wk (+biases)
  - v computed transposed (vT = xd^T @ Wv^T); v-bias commutes through
    softmax (rows sum to 1) and bilinear upsample (rows sum to 1), so
    up = upsample(attn@v) = vT^T @ (attn_n @ KbT) + vb
  - SE global-avg-pool computed from row-sums of xp and of the upsampled
    attention output (accum_out side-channels); SE gate + modality gate
    fused into per-channel affine: out = xp*cw1 + up*cw2
Precision: x/out and residual path bf16; the attention path (v, q/k,
upsample operand) and SE weights fp8 e4m3 with power-of-2 scale folding
(the attention path contributes ~1.4% of output norm, so fp8 error is
strongly attenuated). Matmuls use fp8 DoubleRow (2 k-tiles per pass).
"""

import numpy as np
import ml_dtypes

import concourse.bass as bass
import concourse.tile as tile
import concourse.mybir as mybir

F32 = mybir.dt.float32
BF16 = mybir.dt.bfloat16
F8 = mybir.dt.float8e4
AF = mybir.ActivationFunctionType
ALU = mybir.AluOpType
DR = mybir.MatmulPerfMode.DoubleRow
USE_DR = True              # fp8 DoubleRow for the vt matmuls
USE_DR_QK = True           # fp8 DoubleRow for q/k (full 128-col stationary)
USE_GPSIMD = False         # offload up*cw2 product to the gpsimd engine

N_CORES = 8
B, C, H, W, P = 32, 2048, 48, 24, 3
BL = B // N_CORES          # 4 local batches per core
IC = 128                   # q/k inter channels
C4 = 512                   # SE bottleneck
PH = H // P                # 16
HD, WD = PH // 2, W // 2   # 8, 12
N = HD * WD                # 96 attention tokens
HWP = PH * W               # 384 spatial positions per part
KC = C // 128              # 16 channel tiles
KT = KC // 2               # 8 DoubleRow k-tile pairs
NP2 = 2 * N                # 192 (pair of batches)
NPAD = 256                 # padded pair width for DoubleRow (2 x 128)

# fp8 scale folding (powers of two)
SQ = 4096.0                # q/k weights+biases scale
SV = 64.0                  # Wv scale
SU = 32.0                  # upsampled-attention (upt) scale
SG = 8.0                   # SE gap scale
SF1 = 64.0                 # fc1 scale
SF2 = 64.0                 # fc2 scale


def _up_matrix(n):
    """[2n, n] bilinear x2 upsample (align_corners=False, edge clamp)."""
    M = np.zeros((2 * n, n), np.float64)
    for o in range(2 * n):
        src = (o + 0.5) / 2.0 - 0.5
        i0 = int(np.floor(src))
        f = src - i0
        M[o, min(max(i0, 0), n - 1)] += 1.0 - f
        M[o, min(max(i0 + 1, 0), n - 1)] += f
    return M


def k_bilinear():
    """[384, 96] upsample matrix: flat(16,24) <- flat(8,12)."""
    return np.kron(_up_matrix(HD), _up_matrix(WD))


def split_excess_waits(nc, max_waits=1):
    """This walrus build rejects multi-sem-wait instructions on some opcodes;
    hoist extra waits onto preceding same-engine no-ops."""
    for f in nc.m.functions:
        for bb in f.blocks:
            insts = bb.instructions
            i = 0
            while i < len(insts):
                ins = insts[i]
                si = ins.sync_info
                if si is not None and si.on_wait and len(si.on_wait) > max_waits:
                    waits = list(si.on_wait)
                    extra, keep = waits[:-max_waits], waits[-max_waits:]
                    nops = []
                    for s in range(0, len(extra), max_waits):
                        nops.append(mybir.InstNoOp(
                            name=nc.get_next_instruction_name(),
                            engine=ins.engine, ins=[], outs=[],
                            sync_info=mybir.SyncInfo(
                                on_wait=extra[s:s + max_waits], on_update=[]),
                        ))
                    ins.sync_info = mybir.SyncInfo(
                        on_wait=keep, on_update=list(si.on_update or []))
                    insts[i:i] = nops
                    i += len(nops)
                i += 1


def build_program(split_waits=True):
    from contextlib import ExitStack
    nc = bass.Bass()

    xh = nc.dram_tensor("xh", [C, P, BL * HWP], BF16, kind="ExternalInput")
    wvd = nc.dram_tensor("wvd", [P, KT, 128, 2, C], F8, kind="ExternalInput")
    wqkd = nc.dram_tensor("wqkd", [P, 2, 128, KT, 2, IC], F8,
                          kind="ExternalInput")
    ktd = nc.dram_tensor("ktd", [P, N, HWP + 1], BF16, kind="ExternalInput")
    fc1d = nc.dram_tensor("fc1d", [P, 128, KC, C4], F8, kind="ExternalInput")
    fc2d = nc.dram_tensor("fc2d", [P, 128, 4, C], F8, kind="ExternalInput")
    # consts cols: 0:16 vbg*SU | 16:32 b2 | 32:36 b1*SG*SF1 | 36 qb*SQ | 37 kb*SQ
    consts = nc.dram_tensor("consts", [P, 128, 38], F32, kind="ExternalInput")
    # gg rows: 0 mwc | 1 mw/SU | 2 mwc/SU  (cols = kc*2 + j)
    gg = nc.dram_tensor("gg", [P, 128, 2, 3, 32], F32, kind="ExternalInput")
    outh = nc.dram_tensor("outh", [C, P, BL * HWP], BF16, kind="ExternalOutput")

    sE = 1.0 / (SQ * SQ)

    with ExitStack() as ctx:
        tc = ctx.enter_context(tile.TileContext(nc))
        pool = lambda name, bufs, **kw: ctx.enter_context(
            tc.tile_pool(name=name, bufs=bufs, **kw))
        wv_pool = pool("wv", 12)
        wqk_pool = pool("wqk", 4)
        fc1_pool = pool("fc1", 2)
        fc2_pool = pool("fc2", 2)
        kt_pool = pool("ktp", 2)
        co_pool = pool("co", 2)
        gg_pool = pool("ggp", 2)
        xp_pool = pool("xp", 32)
        xd_pool = pool("xd", 16)
        t1_pool = pool("t1", 4)
        xs_pool = pool("xs", 36)
        us_pool = pool("us", 36)
        qk_pool = pool("qk", 4)
        attn_pool = pool("attn", 4)
        g_pool = pool("gg2", 2)
        vt_pool = pool("vt", 2)
        up_pool = pool("ups", 34)
        gap_pool = pool("gap", 20)
        se_pool = pool("se", 16)
        uc_pool = pool("uc", 4)
        ot_pool = pool("ot", 4)
        sm_pool = pool("sm", 12)
        ps_vt = pool("ps_vt", 1, space="PSUM")
        ps_bank = pool("ps_bank", 4, space="PSUM")

        for p in range(P):
            # ---------- per-part small loads (issued before x of pair 0) ----
            wq_t = wqk_pool.tile([128, KT, 2, IC], F8, tag="wqk")
            nc.sync.dma_start(wq_t[:], wqkd.ap()[p, 0])
            wk_t = wqk_pool.tile([128, KT, 2, IC], F8, tag="wqk")
            nc.sync.dma_start(wk_t[:], wqkd.ap()[p, 1])
            kt_t = kt_pool.tile([N, HWP + 1], BF16, tag="kt")
            nc.sync.dma_start(kt_t[:], ktd.ap()[p])
            co_t = co_pool.tile([128, 38], F32, tag="co")
            nc.sync.dma_start(co_t[:], consts.ap()[p])
            gg_t = gg_pool.tile([128, 2, 3, 32], F32, tag="ggt")
            nc.sync.dma_start(gg_t[:], gg.ap()[p])

            wv_t = [None] * KT
            fc1_t = fc2_t = None

            for pr in range(2):
                # ---------- load x pair, 2x2-sum-pool -> xd (+ row sums) ----
                xp_t, xs_t = [], []
                xd_t = [xd_pool.tile([128, 2, NPAD], F8, tag="xd",
                                     name=f"xd{k}")
                        for k in range(KT)]
                for kt in range(KT):
                    xd4 = xd_t[kt][:].rearrange("q k (j n) -> q k j n", j=2)
                    nc.vector.memset(xd4[:, :, :, N:128], 0.0)
                for kc in range(KC):
                    xt = xp_pool.tile([128, 2 * HWP], BF16, tag="xp")
                    nc.sync.dma_start(
                        xt[:], xh.ap()[kc * 128:(kc + 1) * 128, p,
                                       pr * 2 * HWP:(pr + 1) * 2 * HWP])
                    xp_t.append(xt)
                    xv4 = xt[:].rearrange("q (j h w) -> q j h w", j=2, h=PH)
                    t1 = t1_pool.tile([128, 2, HD, W], BF16, tag="t1")
                    nc.vector.tensor_tensor(
                        t1[:], xv4[:, :, 0:PH:2, :], xv4[:, :, 1:PH:2, :],
                        ALU.add)
                    t1v = t1[:].rearrange("q j h (w two) -> q j h w two", two=2)
                    xs = xs_pool.tile([128, 2], F32, tag="xs")
                    xs_t.append(xs)
                    for j in range(2):
                        nc.vector.scalar_tensor_tensor(
                            xd_t[kc // 2][:, kc % 2, j * 128:j * 128 + N]
                            .rearrange("q (h w) -> q h w", h=HD),
                            t1v[:, j, :, :, 0], 1.0, t1v[:, j, :, :, 1],
                            ALU.mult, ALU.add, accum_out=xs[:, j:j + 1])

                if pr == 0:
                    # wv/fc DMAs issued after pair-0 x loads so x lands first
                    for kt in range(KT):
                        t = wv_pool.tile([128, 2, C], F8, tag="wv")
                        nc.sync.dma_start(t[:], wvd.ap()[p, kt])
                        wv_t[kt] = t
                    fc1_t = fc1_pool.tile([128, KC, C4], F8, tag="fc1")
                    nc.sync.dma_start(fc1_t[:], fc1d.ap()[p])
                    fc2_t = fc2_pool.tile([128, 4, C], F8, tag="fc2")
                    nc.sync.dma_start(fc2_t[:], fc2d.ap()[p])

                # ---------- q/k projections (DoubleRow, batched pair) ----
                q_ps = ps_bank.tile([IC, NPAD], F32, tag="bank")
                if USE_DR_QK:
                    for kt in range(KT):
                        nc.tensor.matmul(q_ps[:], wq_t[:, kt], xd_t[kt][:],
                                         start=(kt == 0), stop=(kt == KT - 1),
                                         perf_mode=DR)
                else:
                    for kc in range(KC):
                        nc.tensor.matmul(q_ps[:], wq_t[:, kc // 2, kc % 2],
                                         xd_t[kc // 2][:, kc % 2],
                                         start=(kc == 0), stop=(kc == KC - 1))
                q_sb = qk_pool.tile([IC, NPAD], BF16, tag="qk")
                nc.scalar.activation(q_sb[:], q_ps[:], AF.Identity,
                                     bias=co_t[:, 36:37])
                k_ps = ps_bank.tile([IC, NPAD], F32, tag="bank")
                if USE_DR_QK:
                    for kt in range(KT):
                        nc.tensor.matmul(k_ps[:], wk_t[:, kt], xd_t[kt][:],
                                         start=(kt == 0), stop=(kt == KT - 1),
                                         perf_mode=DR)
                else:
                    for kc in range(KC):
                        nc.tensor.matmul(k_ps[:], wk_t[:, kc // 2, kc % 2],
                                         xd_t[kc // 2][:, kc % 2],
                                         start=(kc == 0), stop=(kc == KC - 1))
                k_sb = qk_pool.tile([IC, NPAD], BF16, tag="qk")
                nc.scalar.activation(k_sb[:], k_ps[:], AF.Identity,
                                     bias=co_t[:, 37:38])

                us_t = [us_pool.tile([128, 2], F32, tag="us", name=f"us{k}")
                        for k in range(KC)]
                upt_t = {}
                for j in range(2):
                    # ---------- attention (softmax without max-shift:
                    # |energy| ~ 1e-3, exp cannot overflow) ----------
                    e_ps = ps_bank.tile([N, N], F32, tag="bank")
                    nc.tensor.matmul(e_ps[:], q_sb[:, j * 128:j * 128 + N],
                                     k_sb[:, j * 128:j * 128 + N],
                                     start=True, stop=True)
                    attn_e = attn_pool.tile([N, N], BF16, tag="attn")
                    s_sum = sm_pool.tile([N, 1], F32, tag="sm")
                    nc.scalar.activation(attn_e[:], e_ps[:], AF.Exp,
                                         scale=sE, accum_out=s_sum[:])
                    r_sum = sm_pool.tile([N, 1], F32, tag="sm")
                    nc.vector.reciprocal(r_sum[:], s_sum[:])
                    attn_n = attn_pool.tile([N, N], BF16, tag="attn")
                    nc.vector.tensor_scalar(attn_n[:], attn_e[:], r_sum[:],
                                            None, ALU.mult)
                    # ---------- G = attn_n^T @ KbT  [N, 384] ----------
                    g_ps = ps_bank.tile([N, HWP + 1], F32, tag="bank")
                    nc.tensor.matmul(g_ps[:], attn_n[:], kt_t[:],
                                     start=True, stop=True)
                    g_sb = g_pool.tile([N, HWP + 1], BF16, tag="g")
                    nc.scalar.activation(g_sb[:], g_ps[:], AF.Copy)
                    # ---------- vT = xd_b^T @ WvT (DoubleRow) [N, C] ----
                    if USE_DR:
                        vt_ps = ps_vt.tile([128, C], F32, tag="vt")
                        for kt in range(KT):
                            xdb = xd_t[kt][:, :, j * 128:(j + 1) * 128]
                            for bk in range(4):
                                nc.tensor.matmul(
                                    vt_ps[:, bk * 512:(bk + 1) * 512], xdb,
                                    wv_t[kt][:, :, bk * 512:(bk + 1) * 512],
                                    start=(kt == 0), stop=(kt == KT - 1),
                                    perf_mode=DR)
                    else:
                        vt_ps = ps_vt.tile([128, C], F32, tag="vt")
                        for kc in range(KC):
                            xdb = xd_t[kc // 2][:, kc % 2,
                                               j * 128:j * 128 + N]
                            for bk in range(4):
                                nc.tensor.matmul(
                                    vt_ps[0:N, bk * 512:(bk + 1) * 512], xdb,
                                    wv_t[kc // 2][:, kc % 2,
                                                  bk * 512:(bk + 1) * 512],
                                    start=(kc == 0), stop=(kc == KC - 1))
                    vt_sb = vt_pool.tile([N, C], BF16, tag="vt")
                    for bk in range(4):
                        nc.scalar.activation(
                            vt_sb[:, bk * 512:(bk + 1) * 512],
                            vt_ps[0:N, bk * 512:(bk + 1) * 512], AF.Copy)
                    # ---------- up = vT^T @ G (+ SU*gamma*vb); row sums ----
                    for kc in range(KC):
                        up_ps = ps_bank.tile([128, HWP + 1], F32, tag="bank")
                        nc.tensor.matmul(
                            up_ps[:], vt_sb[:, kc * 128:(kc + 1) * 128],
                            g_sb[:], start=True, stop=True)
                        upt = up_pool.tile([128, HWP], BF16, tag="ups")
                        nc.scalar.activation(
                            upt[:], up_ps[:, 0:HWP], AF.Identity,
                            scale=SU / SV, bias=co_t[:, kc:kc + 1])
                        nc.vector.scalar_tensor_tensor(
                            us_t[kc][:, j:j + 1], up_ps[:, HWP:HWP + 1],
                            1.0 / SV, xs_t[kc][:, j:j + 1],
                            ALU.mult, ALU.add)
                        upt_t[(kc, j)] = upt

                # ---------- SE gate (pair, free dim 2) ----------
                gap_t = []
                for kc in range(KC):
                    ga = gap_pool.tile([128, 2], F8, tag="gap")
                    nc.scalar.activation(ga[:], us_t[kc][:], AF.Identity,
                                         scale=SG / HWP)
                    gap_t.append(ga)
                h_ps = [ps_bank.tile([128, 2], F32, tag="bank", name=f"h{m}")
                        for m in range(4)]
                for kc in range(KC):
                    for m in range(4):
                        nc.tensor.matmul(
                            h_ps[m][:], fc1_t[:, kc, m * 128:(m + 1) * 128],
                            gap_t[kc][:], start=(kc == 0), stop=(kc == KC - 1))
                h1_t = []
                for m in range(4):
                    hb = se_pool.tile([128, 2], F8, tag="se")
                    nc.scalar.activation(hb[:], h_ps[m][:], AF.Relu,
                                         bias=co_t[:, 32 + m:33 + m])
                    h1_t.append(hb)
                cw_all = se_pool.tile([128, 32], F32, tag="se")
                for kc in range(KC):
                    c_ps = ps_bank.tile([128, 2], F32, tag="bank")
                    for m in range(4):
                        nc.tensor.matmul(
                            c_ps[:], fc2_t[:, m, kc * 128:(kc + 1) * 128],
                            h1_t[m][:], start=(m == 0), stop=(m == 3))
                    nc.scalar.activation(cw_all[:, 2 * kc:2 * kc + 2], c_ps[:],
                                         AF.Sigmoid, scale=1.0 / (SF2 * SG * SF1),
                                         bias=co_t[:, 16 + kc:17 + kc])
                # cw1 = 1 + mwc*cw ; cw2 = (mw + mwc*cw)/SU
                tmp = se_pool.tile([128, 32], F32, tag="se")
                nc.vector.tensor_tensor(tmp[:], cw_all[:], gg_t[:, pr, 0, :],
                                        ALU.mult)
                cw1 = se_pool.tile([128, 32], F32, tag="se")
                nc.vector.tensor_scalar(cw1[:], tmp[:], 1.0, None, ALU.add)
                tmp2 = se_pool.tile([128, 32], F32, tag="se")
                nc.vector.tensor_tensor(tmp2[:], cw_all[:], gg_t[:, pr, 2, :],
                                        ALU.mult)
                cw2 = se_pool.tile([128, 32], F32, tag="se")
                nc.vector.tensor_tensor(cw2[:], tmp2[:], gg_t[:, pr, 1, :],
                                        ALU.add)

                # ---------- final blend + store ----------
                for kc in range(KC):
                    ot = ot_pool.tile([128, 2 * HWP], BF16, tag="ot")
                    for j in range(2):
                        col = 2 * kc + j
                        r1 = uc_pool.tile([128, HWP], F32, tag="uc")
                        if j == 0:
                            nc.scalar.activation(
                                r1[:], xp_t[kc][:, j * HWP:(j + 1) * HWP],
                                AF.Identity, scale=cw1[:, col:col + 1])
                        else:
                            nc.vector.tensor_scalar(
                                r1[:], xp_t[kc][:, j * HWP:(j + 1) * HWP],
                                cw1[:, col:col + 1], None, ALU.mult)
                        nc.vector.scalar_tensor_tensor(
                            ot[:, j * HWP:(j + 1) * HWP], upt_t[(kc, j)][:],
                            cw2[:, col:col + 1], r1[:], ALU.mult, ALU.add)
                    nc.sync.dma_start(
                        outh.ap()[kc * 128:(kc + 1) * 128, p,
                                  pr * 2 * HWP:(pr + 1) * 2 * HWP], ot[:])

    if split_waits:
        split_excess_waits(nc)
    return nc


# ---------------------------------------------------------------------------
# Host side
# ---------------------------------------------------------------------------

def _sigmoid(v):
    return 1.0 / (1.0 + np.exp(-v))


def _bf(a):
    return np.ascontiguousarray(np.asarray(a).astype(ml_dtypes.bfloat16))


def _f8(a):
    return np.ascontiguousarray(
        np.clip(np.asarray(a, np.float64), -224.0, 224.0)
        .astype(ml_dtypes.float8_e4m3))


def _f32(a):
    return np.ascontiguousarray(np.asarray(a, dtype=np.float32))


def prepare_host_inputs(inputs):
    """Fold/transpose/scale weights; returns per-core input dicts."""
    g = {k: np.asarray(v) for k, v in inputs.items()}
    x = np.asarray(g["x"], np.float32)

    # modality gate on host (tiny): mw [B, P]
    mf = g["modality"].astype(np.float64)[:, None]
    g1 = np.maximum(mf @ g["gate_w1"].astype(np.float64).T
                    + g["gate_b1"].astype(np.float64), 0.0)
    mw = _sigmoid(g1 @ g["gate_w2"].astype(np.float64).T
                  + g["gate_b2"].astype(np.float64))      # [B, P]

    paq = g["pa_q_w"].astype(np.float64)    # [P, IC, C]
    pak = g["pa_k_w"].astype(np.float64)
    pav = g["pa_v_w"].astype(np.float64)    # [P, C, C]
    dwq_w = g["pa_dw_q_w"].astype(np.float64)   # [P, C]
    dwq_b = g["pa_dw_q_b"].astype(np.float64)
    dwk_w = g["pa_dw_k_w"].astype(np.float64)
    dwk_b = g["pa_dw_k_b"].astype(np.float64)
    gam = g["pa_gamma"].astype(np.float64)      # [P]
    cgam = g["ca_gamma"].astype(np.float64)

    # q/k weights [P, C, IC], scaled by SQ (and 0.25 pool fold)
    wq_full = np.stack([(paq[p] * dwq_w[p][None, :] * 0.25 * SQ).T
                        for p in range(P)])
    wk_full = np.stack([(pak[p] * dwk_w[p][None, :] * 0.25 * SQ).T
                        for p in range(P)])
    qb = np.stack([SQ * (g["pa_q_b"][p] + paq[p] @ dwq_b[p]) for p in range(P)])
    kb = np.stack([SQ * (g["pa_k_b"][p] + pak[p] @ dwk_b[p]) for p in range(P)])
    # DoubleRow interleave [P, 2, 128, KT, 2, IC]
    wqk = np.stack([wq_full, wk_full], axis=1)          # [P, 2, C, IC]
    wqkd = _f8(wqk.reshape(P, 2, KT, 2, 128, IC).transpose(0, 1, 4, 2, 3, 5))

    # v weights [P, C, C] scaled by SV -> [P, KT, 128, 2, C]
    wv_full = np.stack([0.25 * SV * pav[p].T for p in range(P)])
    wvd = _f8(wv_full.reshape(P, KT, 2, 128, C).transpose(0, 1, 3, 2, 4))

    # upsample matrix with pa_gamma folded: [P, N, HWP]
    kb_mat = k_bilinear()                     # [384, 96]
    ktd = _bf(np.stack([
        gam[p] * np.concatenate([kb_mat.T, kb_mat.mean(axis=0)[:, None]],
                                axis=1)
        for p in range(P)]))

    fc1 = g["ca_fc1_w"].astype(np.float64)    # [P, C4, C]
    fc2 = g["ca_fc2_w"].astype(np.float64)    # [P, C, C4]
    fc1T = np.stack([SF1 * fc1[p].T for p in range(P)])   # [P, C, C4]
    fc2T = np.stack([SF2 * fc2[p].T for p in range(P)])   # [P, C4, C]
    fc1d = _f8(fc1T.reshape(P, KC, 128, C4).transpose(0, 2, 1, 3))
    fc2d = _f8(fc2T.reshape(P, 4, 128, C).transpose(0, 2, 1, 3))

    consts = np.zeros((P, 128, 38), np.float32)
    for p in range(P):
        vbg = SU * gam[p] * g["pa_v_b"][p].astype(np.float64)     # [C]
        consts[p, :, 0:16] = vbg.reshape(16, 128).T
        consts[p, :, 16:32] = g["ca_fc2_b"][p].astype(np.float64).reshape(
            16, 128).T
        b1f = SG * SF1 * (g["ca_fc1_b"][p].astype(np.float64)
                          + fc1[p] @ (gam[p] * g["pa_v_b"][p].astype(
                              np.float64)))
        consts[p, :, 32:36] = b1f.reshape(4, 128).T
        consts[p, :, 36] = qb[p]
        consts[p, :, 37] = kb[p]

    shared = {"wvd": wvd, "wqkd": wqkd, "ktd": ktd, "fc1d": fc1d,
              "fc2d": fc2d, "consts": consts}

    per_core = []
    for c in range(N_CORES):
        xs = x[c * BL:(c + 1) * BL]            # [BL, C, H, W]
        xhc = _bf(xs.reshape(BL, C, P, PH, W).transpose(1, 2, 0, 3, 4)
                  .reshape(C, P, BL * PH * W))
        mwl = mw[c * BL:(c + 1) * BL]          # [BL, P]
        ggc = np.zeros((P, 128, 2, 3, 32), np.float32)
        for p in range(P):
            for b in range(BL):
                pr, j = b // 2, b % 2
                cols = np.arange(KC) * 2 + j
                mwc = mwl[b, p] * cgam[p]
                ggc[p, :, pr, 0, cols] = mwc
                ggc[p, :, pr, 1, cols] = mwl[b, p] / SU
                ggc[p, :, pr, 2, cols] = mwc / SU
        per_core.append({"xh": xhc, "gg": ggc, **shared})
    return per_core


_CACHE = {}
TRACE = False
TRACE_DIR = None


def kernel(**inputs):
    from concourse.bass_utils import run_bass_kernel_spmd

    per_core = prepare_host_inputs(inputs)
    if "nc" not in _CACHE:
        _CACHE["nc"] = build_program()
    nc = _CACHE["nc"]
    kw = dict(trace=True, tmpdir=TRACE_DIR) if TRACE else {}
    res = run_bass_kernel_spmd(nc, per_core, list(range(N_CORES)), **kw)
    _CACHE["last_results"] = res
    outs = []
    for c in range(N_CORES):
        oh = np.asarray(res.results[c]["outh"]).astype(np.float32)
        outs.append(oh.reshape(C, P, BL, PH, W).transpose(2, 0, 1, 3, 4)
                    .reshape(BL, C, H, W))
    return np.concatenate(outs, axis=0)
